# revision 36
# baseline (speedup 1.0000x reference)
"""Trainium2 Bass kernel for nn_MinimalSSM: selective-scan SSM block.

Reference computation (per batch b):
    proj  = x @ W_xproj + b_xproj                # [L, d+2n]
    delta = softplus(proj[:, :d])                # [L, d]
    Bm, Cm = proj[:, d:d+n], proj[:, d+n:]       # [L, n]
    A     = -exp(A_log)                          # [d, n]
    h_t   = exp(delta_t*A) * h_{t-1} + delta_t*Bm_t*x_t   (elementwise [d, n])
    y_t   = sum_n(h_t * Cm_t) + Dp * x_t
    out   = y @ W_out + b_out

Sharding (8 cores): batch (4) x d_model-half (2).  Each core computes the
full recurrence for its 512 channels of its batch and a partial
out-projection; the two halves of each batch are summed on-device with a
pairwise AllReduce ([[0,4],[1,5],[2,6],[3,7]]) emitted per chunk-pair so
all but the last reduce overlap compute.  Single NEFF, no second stage.

Engine split per core (~400us, DVE-bound at ~310us busy):
  - PE: x-projection, out-projection (deferred one pair, accumulated in
    PSUM), y's n-reduction: diag(Dp)*x plus 8 identity matmuls over
    stride-2 slices of the fused h*C pair-sums.  Strided matmul rhs is
    free (measured 405-417ns for stride 1/2/4 at FD=256).
  - ACT: softplus (Exp+Ln, no Softplus table on TRN2), exp(delta*A)
    per n (scale= is per-partition so n cannot batch; FD=512 pair-wide
    amortizes the ~270ns call bubble), cross-pair carry snapshots,
    PSUM evacuations (GPSIMD cannot read PSUM: walrus rejects it).
  - DVE (custom 2x-mode uops -- see below): dx mult + b-tile mult (stock
    TT already 2x on contiguous bf16, ~0.55ns/el), AFFINE_SCAN2_ANT
    (interleave-2 affine scan, 1x/1.04cyc-per-el -- a 2x scan needs
    quad-interleaved streams whose construction costs more than the scan
    saves), PAIR2X_ANT (interleaves the two C broadcast halves at 2
    el/cyc), MULADD2X_ANT (fused h*C multiply + s-pair-sum at 2 el/cyc,
    replacing two 1x h*C mults + a tree add).
  - GPSIMD: idle (Pool ISA rejects TensorTensorScan; ~3.4ns/el copies are
    too slow to matter; gpsimd.dma_start crashes the device).

Custom 2x ops (the main speedup over the 1x baseline): DveOpSpec supports
uops_2x + perf_max; dve_table_gen 8-aligns those rows and the byte-36
perf_max bits reach the engine via InstCustomDveAnt(perf_max=) -- emit via
_custom_dve_pm below.  2X_1PORT fetches (lo, hi) pairs per port per cycle
(SRC_x + SRC_x_HI input lanes) and MUST write both WR0_LO and WR0_HI every
cycle (one write hangs the engine; duplicate a single result into both
lanes and read downstream at stride 2).  Sources must be contiguous
stride-1 bf16 (middle-dim stride-0 broadcasts still qualify); on
qualification failure the engine silently falls back to the REGULAR slot.
MULADD2X consumes 2N els and emits N dup-pairs; PAIR2X alternates a
consuming uop with a non-consuming one (next_uop index 0 is IDLE, so the
ping-pong lives at uop indices 1/2 with an entry copy at 0; cross-uop
values pass through stage out-flops read back via CURR_ALU_OUT one cycle
later).

Layouts: partition dim = 128-channel block (4 per core); free dims
(n=16, t=TC) per chunk, t innermost.  a-tiles span a chunk pair
[128, 2, N, TC+2] with (0,1) dummy columns so the flat chained scan
self-seeds at every n' boundary; carries inject as b-tile columns 0:2.
h and the C operand are stored interleaved [128, 8*(TC+2), 2] (substream
pairs (n, n+8)); custom-op OUTPUTS must be contiguous, inputs may be
strided views.  Within-pair carries are gathered by the DVE straight from
hti (no ACT round-trip); cross-pair carries snapshot via ACT because hti
buffers recycle.

Measured dead ends, do not revisit blindly: 4-byte-strided DMA
destinations run at ~6.8ns per 2-byte beat (the interleave must happen on
an engine, not in a DMA pattern); DMA access patterns are limited to 3
levels; collectives cannot read or write ExternalInput/Output tensors
(stage through internal DRAM); a 0.5MB pairwise AllReduce costs ~10-15us;
ACT strided (stride-4) writes cost 2.8x contiguous, which kills
quad-interleaved scan-stream construction; batching the per-n exp via a
precomputed delta*A product just moves the same elements onto the DVE
(zero-sum); PSUM-pair softplus batching fails because bias= is
per-partition.
"""

import numpy as np
import ml_dtypes

import concourse.bacc as bacc
import concourse.bass as bass
import concourse.tile as tile
from concourse import mybir
from concourse.bass_utils import run_bass_kernel_spmd
from contextlib import ExitStack

F32 = mybir.dt.float32
BF16 = mybir.dt.bfloat16
FP16 = mybir.dt.float16
AF = mybir.ActivationFunctionType
OP = mybir.AluOpType

B, T, D, N = 4, 2048, 1024, 16
DL = D // 2          # channels per core
NJB = DL // 128      # 4 local channel blocks
NKB = D // 128       # 8 contraction blocks for proj
NEB = D // 128       # 8 output-column blocks
PC = 544             # proj columns per core: 512 delta + 16 B + 16 C
TC = 256             # time chunk
TCP = TC + 2
NCH = T // TC
NPAIR = NCH // 2
TP = 2 * TC          # chunk-pair width



# --- custom DVE op: interleave-2 affine scan (1 elem/cycle) ---


from dataclasses import dataclass

import numpy as np

import concourse.dve_ops as dve_ops
from concourse.dve_ops import DveOp, get_dve_sub_opcode
from concourse.dve_spec import (
    Spec, Src0, Src1, AluOp, AluInp, UopConfig, UopDpConfig,
)
from concourse.dve_uop import DveOpSpec
from concourse.dve_uop import DelayInp, InpSel, OutPath, OutSel, Trigger

N_STAGES = 8


def _dp(stage0_op, stage0_src1):
    """Datapath: s0 = stage0_op(lane0=a, stage0_src1); s1 = ADD(prev, lane1=b)
    with out_a enabled; s2..s7 bypass."""
    dps = []
    for st in range(N_STAGES):
        if st == 0:
            cfg = UopDpConfig(
                op=stage0_op,
                alu_src0=AluInp.PREV_DELAY_0,
                alu_src1=stage0_src1,
                delay=[DelayInp.PREV_DELAY] * 3 + [DelayInp.PREV_ALU_OUT] * 4,
                alu_out_enable=1, swap_enable=0,
                alu_out_a_enable=0, alu_out_b_enable=0,
                delay_enable=[1, 1, 0, 0, 0, 0, 0],
                idx0_sel=0, idx1_sel=0,
            )
        elif st == 1:
            cfg = UopDpConfig(
                op=AluOp.ADD,
                alu_src0=AluInp.PREV_ALU_OUT,
                alu_src1=AluInp.PREV_DELAY_1,
                delay=[DelayInp.PREV_DELAY] * 3 + [DelayInp.PREV_ALU_OUT] * 4,
                alu_out_enable=1, swap_enable=0,
                alu_out_a_enable=1, alu_out_b_enable=0,
                delay_enable=[0, 0, 0, 0, 0, 0, 0],
                idx0_sel=0, idx1_sel=0,
            )
        else:
            cfg = UopDpConfig(
                op=AluOp.BYPASS,
                alu_src0=AluInp.PREV_ALU_OUT,
                alu_src1=AluInp.PREV_ALU_OUT,
                delay=[DelayInp.PREV_DELAY] * 3 + [DelayInp.PREV_ALU_OUT] * 4,
                alu_out_enable=1, swap_enable=0,
                alu_out_a_enable=0, alu_out_b_enable=0,
                delay_enable=[0, 0, 0, 0, 0, 0, 0],
                idx0_sel=0, idx1_sel=0,
            )
        dps.append(cfg)
    return dps


def _uop(dp, *, write_out, trigger, next_uop, repeat):
    inp = [InpSel.ZERO] * 8
    inp_enable = [0] * 8
    inp[1], inp_enable[1] = InpSel.SRC_0, 1      # lane0 = a
    inp[2], inp_enable[2] = InpSel.SRC_1, 1      # lane1 = b
    out = {o: OutSel.ALU_OUT for o in OutPath}
    out_enable = {o: 0 for o in OutPath}
    if write_out:
        out_enable[OutPath.WR0_LO] = 1
    return UopConfig(
        datapath_config=dp,
        inp=inp,
        inp_enable=inp_enable,
        out=out,
        out_enable=out_enable,
        accum_enabled=0,
        require_inp0=1,
        require_inp1=1,
        trigger=trigger,
        next_uop=next_uop,
        repeat_count=repeat,
    )


def _build_uops():
    # seed (2 elements): m = a (bypass, no flop read); h = m + b
    seed = _uop(
        _dp(AluOp.BYPASS, AluInp.PREV_DELAY_0),
        write_out=True,
        trigger=(Trigger.COUNT, Trigger.NONE, Trigger.NONE),
        next_uop=(1, 0, 0),
        repeat=2,
    )
    # steady: m = a * h_fb (lag-2 via stage1's out_a flop); h = m + b
    steady = _uop(
        _dp(AluOp.MULTIPLY, AluInp.NEXT_ALU_OUT_A),
        write_out=True,
        trigger=(Trigger.SRC_TENSOR_DONE, Trigger.NONE, Trigger.NONE),
        next_uop=(0, 0, 0),
        repeat=0,
    )
    return [seed, steady]


def _reference(in0, in1, s0, s1, imm2):
    a = np.asarray(in0, np.float32)
    b = np.asarray(in1, np.float32)
    shp = a.shape
    a = a.reshape(shp[0], -1)
    b = b.reshape(shp[0], -1)
    h = np.empty_like(a)
    h[:, 0] = b[:, 0]
    h[:, 1] = b[:, 1]
    for e in range(2, a.shape[1]):
        h[:, e] = a[:, e] * h[:, e - 2] + b[:, e]
    return h.reshape(shp)


@dataclass(frozen=True)
class RawDveOp(DveOp):
    """DveOp whose uop program is hand-written rather than lower()ed."""

    def compile(self, ver):
        key = (self.name, ver)
        cache = dve_ops._COMPILE_CACHE
        if (r := cache.get(key)) is not None:
            return r
        result = DveOpSpec(
            name=self.name,
            opcode=get_dve_sub_opcode(self.name),
            uops=_build_uops(),
            rd1_en=True,
        )
        cache[key] = result
        return result


SCAN2 = RawDveOp(
    "AFFINE_SCAN2_ANT",
    Spec(body=Src0 * Src1, reference=_reference),
    subdim=False,
    uops_sha={},
)


# --- custom DVE op: 2x fused multiply + pair-sum (dup-written) ---
# out pair e: s = a_lo*b_lo + a_hi*b_hi, written to BOTH lanes (the 2x
# write path requires LO+HI writes each cycle; one-write hangs the engine).
# Runs in 2X_1PORT mode (perf_max=1): consumes 2 els/port/cycle.

from concourse.dve_uop import (
    DveOpSpec as _DveOpSpec2, UopConfig as _UopConfig2,
    UopDpConfig as _UopDpConfig2, AluOp as _AluOp2, AluInp as _AluInp2,
    DelayInp as _DelayInp2, InpSel as _InpSel2, OutPath as _OutPath2,
    OutSel as _OutSel2, Trigger as _Trigger2,
)


def _muladd2x_uops():
    import copy as _copy
    dps = [_UopDpConfig2() for _ in range(8)]
    dps[0].enable_alu(_AluOp2.MULTIPLY, _AluInp2.PREV_ALU_OUT,
                      _AluInp2.PREV_DELAY_0)
    dps[0].pass_through_delay(1, 2)
    dps[1].enable_alu(_AluOp2.MULTIPLY, _AluInp2.PREV_DELAY_1,
                      _AluInp2.PREV_DELAY_2)
    dps[1].enable_delay_from_src(_DelayInp2.PREV_ALU_OUT, 0)
    dps[2].enable_alu(_AluOp2.ADD, _AluInp2.PREV_ALU_OUT,
                      _AluInp2.PREV_DELAY_0)
    for st in range(3, 8):
        dps[st].pass_through_alu()
    u = _UopConfig2(datapath_config=dps)
    u.enable_input(_InpSel2.SRC_0, 0)
    u.enable_input(_InpSel2.SRC_1, 1)
    u.enable_input(_InpSel2.SRC_0_HI, 2)
    u.enable_input(_InpSel2.SRC_1_HI, 3)
    u.enable_output(_OutSel2.ALU_OUT, _OutPath2.WR0_LO)
    u.enable_output(_OutSel2.ALU_OUT, _OutPath2.WR0_HI)
    u.require_inp0 = 1
    u.require_inp1 = 1
    u.trigger = (_Trigger2.SRC_TENSOR_DONE, _Trigger2.NONE, _Trigger2.NONE)
    return [_copy.deepcopy(u)], [u]


def _pair2x_uops():
    """Interleave two contiguous streams at 2 els/cycle:
    out = (x0, y0, x1, y1, ...) from in0 = (x0, x1, ...), in1 = (y0, y1...).
    uopA consumes a pair from each port, emits (x_lo, y_lo), stashes the HI
    elements in stage out-flops; uopB (non-consuming) emits them via
    CURR_ALU_OUT self-reads."""
    import copy as _copy
    # uopA
    dps = [_UopDpConfig2() for _ in range(8)]
    dps[0].enable_alu(_AluOp2.BYPASS, _AluInp2.PREV_ALU_OUT,
                      _AluInp2.PREV_ALU_OUT)          # flop0 = x_lo
    dps[0].pass_through_delay(0, 1, 2)
    dps[1].enable_alu(_AluOp2.BYPASS, _AluInp2.PREV_DELAY_1,
                      _AluInp2.PREV_DELAY_1)          # flop1 = x_hi (stash)
    dps[1].enable_delay_from_src(_DelayInp2.PREV_ALU_OUT, 3)  # lane3 = x_lo
    dps[1].pass_through_delay(0, 2)
    dps[2].enable_alu(_AluOp2.BYPASS, _AluInp2.PREV_DELAY_2,
                      _AluInp2.PREV_DELAY_2)          # flop2 = y_hi (stash)
    dps[2].pass_through_delay(0, 3)
    for st in range(3, 8):
        dps[st].pass_through_delay(0, 3)
    ua = _UopConfig2(datapath_config=dps)
    ua.enable_input(_InpSel2.SRC_0, 0)
    ua.enable_input(_InpSel2.SRC_1, 1)
    ua.enable_input(_InpSel2.SRC_0_HI, 2)
    ua.enable_input(_InpSel2.SRC_1_HI, 3)
    ua.enable_output(_OutSel2.DELAY_3, _OutPath2.WR0_LO)   # x_lo
    ua.enable_output(_OutSel2.DELAY_0, _OutPath2.WR0_HI)   # y_lo
    ua.require_inp0 = 1
    ua.require_inp1 = 1
    ua.trigger = (_Trigger2.COUNT, _Trigger2.NONE, _Trigger2.NONE)
    ua.next_uop = (1, 0, 0)
    ua.repeat_count = 1
    # uopB: no consume; read the stashed flops
    dps = [_UopDpConfig2() for _ in range(8)]
    dps[1].enable_alu(_AluOp2.BYPASS, _AluInp2.CURR_ALU_OUT,
                      _AluInp2.CURR_ALU_OUT)          # x_hi
    dps[2].enable_alu(_AluOp2.BYPASS, _AluInp2.CURR_ALU_OUT,
                      _AluInp2.CURR_ALU_OUT)          # y_hi
    dps[2].enable_delay_from_src(_DelayInp2.PREV_ALU_OUT, 0)  # lane0 = x_hi
    dps[3].enable_delay_from_src(_DelayInp2.PREV_ALU_OUT, 1)  # lane1 = y_hi
    dps[3].pass_through_delay(0)
    for st in range(4, 8):
        dps[st].pass_through_delay(0, 1)
    ub = _UopConfig2(datapath_config=dps)
    ub.enable_output(_OutSel2.DELAY_0, _OutPath2.WR0_LO)   # x_hi
    ub.enable_output(_OutSel2.DELAY_1, _OutPath2.WR0_HI)   # y_hi
    ub.require_inp0 = 0
    ub.require_inp1 = 0
    # next_uop index 0 always means IDLE, so the A<->B loop lives at
    # indices 1 and 2; index 0 is an entry copy of A.
    ub.trigger = (_Trigger2.SRC_TENSOR_DONE, _Trigger2.COUNT, _Trigger2.NONE)
    ub.next_uop = (0, 1, 0)
    ub.repeat_count = 1
    ua.next_uop = (2, 0, 0)
    ua0 = _copy.deepcopy(ua)
    uops = [ua0, ua, ub]
    # REGULAR fallback: same program (only reachable if 2x qualification
    # fails; numerics check catches that).
    return [_copy.deepcopy(u) for u in uops], uops


def _ref_muladd2(in0, in1, s0, s1, imm2):
    a = np.asarray(in0, np.float32)
    b = np.asarray(in1, np.float32)
    a = a.reshape(a.shape[0], -1)
    b = b.reshape(b.shape[0], -1)
    s = a[:, 0::2] * b[:, 0::2] + a[:, 1::2] * b[:, 1::2]
    return np.repeat(s, 2, axis=1)


def _ref_pair2(in0, in1, s0, s1, imm2):
    a = np.asarray(in0, np.float32)
    b = np.asarray(in1, np.float32)
    a = a.reshape(a.shape[0], -1)
    b = b.reshape(b.shape[0], -1)
    out = np.empty((a.shape[0], 2 * a.shape[1]), np.float32)
    out[:, 0::2] = a
    out[:, 1::2] = b
    return out


_UOP_BUILDERS = {
    "MULADD2X_ANT": _muladd2x_uops,
    "PAIR2X_ANT": _pair2x_uops,
}


@dataclass(frozen=True)
class Raw2xDveOp(DveOp):
    """Hand-written uop program with a 2x perf-mode variant."""

    def compile(self, ver):
        key = (self.name, ver)
        cache = dve_ops._COMPILE_CACHE
        if (r := cache.get(key)) is not None:
            return r
        uops_1x, uops_2x = _UOP_BUILDERS[self.name]()
        result = _DveOpSpec2(
            name=self.name,
            opcode=get_dve_sub_opcode(self.name),
            uops=uops_1x,
            uops_2x=uops_2x,
            rd1_en=True,
            perf_max=1,
        )
        cache[key] = result
        return result


MULADD2X = Raw2xDveOp(
    "MULADD2X_ANT",
    Spec(body=Src0 * Src1, reference=_ref_muladd2),
    subdim=False,
    uops_sha={},
)

PAIR2X = Raw2xDveOp(
    "PAIR2X_ANT",
    Spec(body=Src0 * Src1, reference=_ref_pair2),
    subdim=False,
    uops_sha={},
)


def register():
    for op in (SCAN2, MULADD2X, PAIR2X):
        if op.name in dve_ops._SUB_OPCODE_FOR_NAME:
            continue
        row = max(dve_ops._SUB_OPCODE_FOR_NAME.values()) + 1
        assert row < 0x20
        dve_ops._SUB_OPCODE_FOR_NAME[op.name] = row
        dve_ops.OPS.append(op)
        dve_ops.CUSTOM_DVE_SPECS[op.name] = op.spec
    return SCAN2


def _custom_dve_pm(nc, op, out, in0, in1):
    """Like nc.vector._custom_dve but carries the spec's perf_max into the
    instruction (byte-36 bits 7:6), enabling the 2x uop program."""
    import concourse.bass_isa as bass_isa
    eng = nc.vector
    if op.name not in eng.bass.m.ant_custom_dve_ops:
        eng.bass.m.ant_custom_dve_ops = sorted(
            {*eng.bass.m.ant_custom_dve_ops, op.name})
    compiled = op.compile("v3")
    shape = (bass_isa.CustomDveShape.STT if len(in1.shape) > 2
             else bass_isa.CustomDveShape.TTSS)
    isa_opcode = eng.bass.isa.Opcode[
        f"NEURON_ISA_TPB_OPCODE_CUSTOM_DVE_ANT_{shape.slot()}"].value
    ins = [eng.lower_ap(in0, for_isa=True, opt=True),
           eng.lower_ap(in1, for_isa=True, opt=True),
           mybir.ImmediateValue(dtype=mybir.dt.float32, value=0.0),
           mybir.ImmediateValue(dtype=mybir.dt.float32, value=0.0)]
    outs = [eng.lower_ap(out, for_isa=True, opt=True)]
    return eng.add_instruction(bass_isa.InstCustomDveAnt(
        name=eng.bass.get_next_instruction_name(),
        op_name=op.name,
        rd1_en=True,
        subdim=0,
        imm2=0.0,
        shape=shape,
        row=get_dve_sub_opcode(op.name),
        isa_opcode=isa_opcode,
        ins=ins,
        outs=outs,
        perf_max=compiled.perf_max,
    ))


USE_SCAN2 = True

_cache = {}


def _pin_act_tables():
    """Restrict bacc's activation-table choices to the one set containing
    every function we use (Exp, Ln, Identity, MemsetZero) so the compiler
    never inserts mid-kernel ACT_TABLE_LOAD switches."""
    import concourse.bacc as _bacc_mod
    from concourse.hw_specs import get_activation_tables as _orig

    def _only_nl_exp(arch):
        tabs = _orig(arch)
        return {k: (v if k == "natural_log_exp_and_others" else set())
                for k, v in tabs.items()}

    _bacc_mod.get_activation_tables = _only_nl_exp


_pin_act_tables()


def _build_stage1(t_len=T):
    register()
    nc = bacc.Bacc("TRN2", target_bir_lowering=False, debug=False, num_devices=8)
    xt = nc.dram_tensor("xt", [D, t_len], BF16, kind="ExternalInput")
    wx = nc.dram_tensor("wx", [D, PC], BF16, kind="ExternalInput")
    bx = nc.dram_tensor("bx", [128, 5], F32, kind="ExternalInput")
    alog = nc.dram_tensor("alog", [128, NJB * N], F32, kind="ExternalInput")
    dpdiag = nc.dram_tensor("dpdiag", [NJB * 128, 128], BF16,
                            kind="ExternalInput")
    wo = nc.dram_tensor("wo", [DL, D], BF16, kind="ExternalInput")
    bo = nc.dram_tensor("bo", [128, NEB], F32, kind="ExternalInput")
    ident = nc.dram_tensor("ident", [128, 128], BF16, kind="ExternalInput")
    part = nc.dram_tensor("part", [D, t_len], FP16, kind="ExternalOutput")
    npair_ = t_len // TP
    part_stage = nc.dram_tensor("part_stage", [npair_, 2, D, TC], FP16)
    part_red = nc.dram_tensor("part_red", [npair_, 2, D, TC], FP16)
    bc_dram = nc.dram_tensor("bc_scratch", [NPAIR, 32, TP], BF16)

    npair = t_len // TP

    with tile.TileContext(nc) as tc_ctx, ExitStack() as ctx:
        const = ctx.enter_context(tc_ctx.tile_pool(name="const", bufs=1))
        pjpool = ctx.enter_context(
            tc_ctx.tile_pool(name="pj", bufs=3, space="PSUM"))
        ypspool = ctx.enter_context(
            tc_ctx.tile_pool(name="yps", bufs=2, space="PSUM"))
        popsum = ctx.enter_context(
            tc_ctx.tile_pool(name="pops", bufs=2, space="PSUM"))
        dpool = ctx.enter_context(tc_ctx.tile_pool(name="delta", bufs=3))
        dxpool = ctx.enter_context(tc_ctx.tile_pool(name="dx", bufs=2))
        reppool = ctx.enter_context(tc_ctx.tile_pool(name="rep", bufs=2))
        cintpool = ctx.enter_context(tc_ctx.tile_pool(name="cint", bufs=2))
        apool = ctx.enter_context(tc_ctx.tile_pool(name="apool", bufs=2))
        workpool = ctx.enter_context(tc_ctx.tile_pool(name="work", bufs=2))
        hpool = ctx.enter_context(tc_ctx.tile_pool(name="hpool", bufs=2))
        ybfpool = ctx.enter_context(tc_ctx.tile_pool(name="ybf", bufs=2))
        popool = ctx.enter_context(tc_ctx.tile_pool(name="po", bufs=4))

        bx_sb = const.tile([128, 5], F32, tag="bx")
        nc.sync.dma_start(bx_sb[:], bx[:])
        alog_sb = const.tile([128, NJB * N], F32, tag="alog")
        nc.sync.dma_start(alog_sb[:], alog[:])
        xt_sb = []
        for kb in range(NKB):
            tt = const.tile([128, t_len], BF16, tag=f"xt{kb}")
            nc.sync.dma_start(tt[:, 0:TP], xt[kb * 128:(kb + 1) * 128, 0:TP])
            xt_sb.append(tt)
        wx_sb = []
        for kb in range(NKB):
            tt = const.tile([128, PC], BF16, tag=f"wx{kb}")
            nc.sync.dma_start(tt[:], wx[kb * 128:(kb + 1) * 128, :])
            wx_sb.append(tt)
        for kb in range(NKB):
            nc.sync.dma_start(xt_sb[kb][:, TP:],
                              xt[kb * 128:(kb + 1) * 128, TP:])
        wo_sb = []
        for kb in range(NJB):
            tt = const.tile([128, D], BF16, tag=f"wo{kb}")
            nc.sync.dma_start(tt[:], wo[kb * 128:(kb + 1) * 128, :])
            wo_sb.append(tt)
        dpd_sb = []
        for jb in range(NJB):
            tt = const.tile([128, 128], BF16, tag=f"dpd{jb}")
            nc.sync.dma_start(tt[:], dpdiag[jb * 128:(jb + 1) * 128, :])
            dpd_sb.append(tt)
        bo_sb = const.tile([128, NEB], F32, tag="bo")
        nc.sync.dma_start(bo_sb[:], bo[:])
        aexp_sb = const.tile([128, NJB * N], F32, tag="aexp")
        nc.scalar.activation(aexp_sb[:], alog_sb[:], AF.Exp)
        aneg_sb = const.tile([128, NJB * N], F32, tag="aneg")
        nc.vector.tensor_scalar_mul(aneg_sb[:], aexp_sb[:], -1.0)
        id_sb = const.tile([128, 128], BF16, tag="ident")
        nc.sync.dma_start(id_sb[:], ident[:])
        # (0, 1) bf16 pattern for the chained-scan dummy columns, per
        # (chunk-in-pair, n)
        const01 = const.tile([128, 2, N, 2], BF16, tag="const01")
        nc.vector.memset(const01[:, :, :, 0:1], 0.0)
        nc.vector.memset(const01[:, :, :, 1:2], 1.0)
        # persistent carry ring: column 1 stays zero forever, so injection
        # is a single [128, N, 2] copy instead of copy+memset
        carry_ring = []
        for i in range(10):
            car = const.tile([128, N, 2], BF16, tag=f"car{i}")
            nc.vector.memset(car[:], 0.0)
            carry_ring.append(car)
        carry_idx = [0]

        def next_carry():
            t = carry_ring[carry_idx[0] % len(carry_ring)]
            carry_idx[0] += 1
            return t

        carry_tiles = [None] * NJB
        pending_out = None

        for cp in range(npair):
            t0 = cp * TP
            # --- projection for the whole pair, FD=512 ---
            delta_tiles = []
            bct = None
            for m in (0, 4, 1, 2, 3):   # delta-0 first: unblocks jb0's exps
                mm = 128 if m < 4 else 32
                ps = pjpool.tile([mm, TP], F32, tag="pj")
                for kb in range(NKB):
                    nc.tensor.matmul(
                        ps[:],
                        wx_sb[kb][:, m * 128:m * 128 + mm],
                        xt_sb[kb][:, t0:t0 + TP],
                        start=(kb == 0), stop=(kb == NKB - 1))
                if m < 4:
                    # softplus(v) = ln(1 + exp(v)); Exp+Ln share one ACT table
                    et_ = dpool.tile([128, TP], F32, tag="etmp")
                    nc.scalar.activation(et_[:], ps[:], AF.Exp,
                                         bias=bx_sb[:, m:m + 1])
                    dt_ = dpool.tile([128, TP], BF16, tag="delta")
                    nc.scalar.activation(dt_[:], et_[:], AF.Ln, bias=1.0)
                    delta_tiles.append(dt_)
                else:
                    bct = dpool.tile([32, TP], BF16, tag="bc")
                    nc.scalar.activation(bct[:], ps[:], AF.Identity,
                                         bias=bx_sb[:32, 4:5])
            nc.sync.dma_start(bc_dram[cp], bct[:])
            breps, crep_ints = [], []
            for ch in range(2):
                brep = reppool.tile([128, N, TC], BF16, tag="brep")
                nc.sync.dma_start(
                    brep[:], bc_dram[cp, 0:N, ch * TC:(ch + 1) * TC]
                    .partition_broadcast(128))
                breps.append(brep)
                # two dummy-padded n-major halves, contiguous DMA writes
                crep = reppool.tile([128, 2, 8, TCP], BF16, tag="crep")
                for s in range(2):
                    nc.sync.dma_start(
                        crep[:, s, :, 2:],
                        bc_dram[cp, N + 8 * s:N + 8 * s + 8,
                                ch * TC:(ch + 1) * TC]
                        .partition_broadcast(128))
                # (u, s)-interleaved C matching hti's stream layout, built
                # by the PAIR2X custom op at 2 els/cycle on the DVE itself
                # (same queue as its consumers: no cross-engine stalls)
                cint = cintpool.tile([128, 8 * TCP, 2], BF16, tag="cint")
                _custom_dve_pm(
                    nc, PAIR2X,
                    cint[:].rearrange("p u s -> p (u s)"),
                    crep[:, 0].rearrange("p a t -> p (a t)"),
                    crep[:, 1].rearrange("p a t -> p (a t)"))
                crep_ints.append(cint)

            # --- previous pair's out-projection (deferred off critical path) ---
            if pending_out is not None:
                pybf, pt0 = pending_out
                pcp = pt0 // TP
                for eb in range(NEB):
                    pso = popsum.tile([128, TP], F32, tag="po")
                    for kb in range(NJB):
                        nc.tensor.matmul(
                            pso[:],
                            wo_sb[kb][:, eb * 128:(eb + 1) * 128],
                            pybf[kb][:],
                            start=(kb == 0), stop=(kb == NJB - 1))
                    pot = popool.tile([128, TP], FP16, tag="pot")
                    nc.scalar.activation(pot[:], pso[:], AF.Identity,
                                         bias=bo_sb[:, eb:eb + 1])
                    nc.sync.dma_start(
                        part_stage[pcp, :, eb * 128:(eb + 1) * 128, :]
                        .rearrange("c p t -> p c t"), pot[:])
                # pairwise cross-core sum of this pair's partials, then copy
                # into the output; all but the last pair overlap compute
                for pch in range(2):
                    nc.gpsimd.collective_compute(
                        "AllReduce",
                        mybir.AluOpType.add,
                        replica_groups=[[0, 4], [1, 5], [2, 6], [3, 7]],
                        ins=[part_stage[pcp, pch].opt()],
                        outs=[part_red[pcp, pch].opt()],
                    )
                    nc.sync.dma_start(
                        part[:, pt0 + pch * TC:pt0 + (pch + 1) * TC],
                        part_red[pcp, pch])

            # --- recurrence, block-outer so each a-tile is consumed before
            # two more are live (apool bufs=2), chunks inner for the carry ---
            ybf2 = []
            for j in range(NJB):
                ybt = ybfpool.tile([128, TP], BF16, tag=f"ybf{j}")
                ybf2.append(ybt)
            pending_y = None

            def _emit_y(pjb, pch, phcs, pct0):
                # phcs: [128, 8*TCP, 2] dup-written pair-sums; PE reads the
                # even lane at stride 2 (measured: same cost as contiguous)
                yps = ypspool.tile([128, TC], F32, tag="yps")
                nc.tensor.matmul(yps[:], dpd_sb[pjb][:],
                                 xt_sb[pjb][:, pct0:pct0 + TC],
                                 start=True, stop=False)
                for n in range(8):
                    nc.tensor.matmul(yps[:], id_sb[:],
                                     phcs[:, n * TCP + 2:(n + 1) * TCP, 0],
                                     start=False, stop=(n == 7))
                nc.scalar.activation(ybf2[pjb][:, pch * TC:(pch + 1) * TC],
                                     yps[:], AF.Identity)
            for jb in range(NJB):
                dt_ = delta_tiles[jb]
                at2 = apool.tile([128, 2, N, TCP], BF16, tag="a")
                nc.vector.tensor_copy(at2[:, :, :, 0:2], const01[:])
                for n in range(N):
                    nc.scalar.activation(
                        at2[:, :, n, 2:], dt_[:], AF.Exp,
                        scale=aneg_sb[:, jb * N + n:jb * N + n + 1])
                dxt = dxpool.tile([128, TP], BF16, tag="dx")
                nc.vector.tensor_mul(dxt[:], dt_[:], xt_sb[jb][:, t0:t0 + TP])

                # b-tiles for both chunks up front (GPSIMD, cols 2:) so the
                # GP mults overlap the DVE scans; carries go in just in time
                bts = []
                for ch in range(2):
                    bt = workpool.tile([128, N, TCP], BF16, tag="work")
                    dx_b = (dxt[:, ch * TC:(ch + 1) * TC]
                            .unsqueeze(1).broadcast_to([128, N, TC]))
                    nc.vector.tensor_mul(bt[:, :, 2:], dx_b, breps[ch][:])
                    bts.append(bt)
                htis = [None, None]
                for ch in range(2):
                    ct0 = t0 + ch * TC
                    bt = bts[ch]
                    if cp == 0 and ch == 0:
                        nc.vector.tensor_copy(bt[:, :, 0:2], carry_ring[-1][:])
                    elif ch == 1:
                        # within-pair carry read straight from hti(ch0) on
                        # the DVE queue (no cross-engine extraction)
                        nc.vector.tensor_copy(bt[:, :, 1:2], carry_ring[-1][:, :, 1:2])
                        nc.vector.tensor_copy(
                            bt[:, :, 0],
                            htis[0][:].rearrange("p (a t) s -> p a t s", a=8)
                            [:, :, TCP - 1, :].rearrange("p a s -> p s a"))
                    else:
                        nc.vector.tensor_copy(bt[:, :, 0:2], carry_tiles[jb][:])
                    if True:
                        # h stored INTERLEAVED: [p, u=(n' 8, t 258), s=n-half]
                        # so the custom scan's output is a contiguous stream
                        # (strided custom-op writes misbehave).
                        hti = hpool.tile([128, 8 * TCP, 2], BF16, tag="h")
                        htis[ch] = hti
                        fa = at2[:, ch].rearrange("p n t -> p (n t)")
                        fb = bt[:].rearrange("p n t -> p (n t)")
                        nc.vector._custom_dve(
                            SCAN2,
                            out=hti[:],
                            in0=fa.rearrange("p (s u) -> p u s", s=2),
                            in1=fb.rearrange("p (s u) -> p u s", s=2))
                        if ch == 1 and cp != npair - 1:
                            # cross-pair carry still snapshots via ACT
                            # (hti does not survive to the next pair)
                            newc = next_carry()
                            nc.scalar.activation(
                                newc[:, :, 0].rearrange("p (s a) -> p a s", s=2),
                                hti[:].rearrange("p (a t) s -> p a t s",
                                                 a=8)[:, :, TCP - 1, :],
                                AF.Identity)
                            carry_tiles[jb] = newc
                        # fused h*C + s-pair-sum: single 2x custom op over the
                        # interleaved streams; sums dup-written to both lanes.
                        # Dummy cols multiply garbage C but are never read.
                        hcs = hpool.tile([128, 8 * TCP, 2], BF16, tag="h")
                        _custom_dve_pm(
                            nc, MULADD2X,
                            hcs[:].rearrange("p u s -> p (u s)"),
                            hti[:].rearrange("p u s -> p (u s)"),
                            crep_ints[ch][:].rearrange("p u s -> p (u s)"))
                    # 8 n-slices + diag(Dp) accumulate on PE
                    _emit_y(jb, ch, hcs, ct0)

            pending_out = (ybf2, t0)

        # final out-projection split per chunk so the ch0 half overlaps the
        # last chunk's scans
        pybf, pt0 = pending_out
        pcp = pt0 // TP
        for pch in range(2):
            for eb in range(NEB):
                pso = popsum.tile([128, TC], F32, tag="po")
                for kb in range(NJB):
                    nc.tensor.matmul(
                        pso[:],
                        wo_sb[kb][:, eb * 128:(eb + 1) * 128],
                        pybf[kb][:, pch * TC:(pch + 1) * TC],
                        start=(kb == 0), stop=(kb == NJB - 1))
                pot = popool.tile([128, TC], FP16, tag="pot")
                nc.scalar.activation(pot[:], pso[:], AF.Identity,
                                     bias=bo_sb[:, eb:eb + 1])
                nc.sync.dma_start(
                    part_stage[pcp, pch, eb * 128:(eb + 1) * 128, :], pot[:])
            # ch0's AllReduce launches while ch1's out-projection still runs
            nc.gpsimd.collective_compute(
                "AllReduce",
                mybir.AluOpType.add,
                replica_groups=[[0, 4], [1, 5], [2, 6], [3, 7]],
                ins=[part_stage[pcp, pch].opt()],
                outs=[part_red[pcp, pch].opt()],
            )
            nc.sync.dma_start(
                part[:, pt0 + pch * TC:pt0 + (pch + 1) * TC],
                part_red[pcp, pch])
    nc.compile()
    return nc


def _build_stage2(t_len=T):
    nc = bacc.Bacc("TRN2", target_bir_lowering=False, debug=False, num_devices=8)
    p0 = nc.dram_tensor("p0", [DL, t_len], FP16, kind="ExternalInput")
    p1 = nc.dram_tensor("p1", [DL, t_len], FP16, kind="ExternalInput")
    s = nc.dram_tensor("s", [DL, t_len], FP16, kind="ExternalOutput")
    tcw = 2048
    with tile.TileContext(nc) as tc_ctx, ExitStack() as ctx:
        pool = ctx.enter_context(tc_ctx.tile_pool(name="p", bufs=6))
        for kb in range(DL // 128):
            for i in range(t_len // tcw):
                t0 = i * tcw
                a_t = pool.tile([128, tcw], FP16, tag="a")
                nc.sync.dma_start(a_t[:], p0[kb * 128:(kb + 1) * 128, t0:t0 + tcw])
                b_t = pool.tile([128, tcw], FP16, tag="b")
                nc.sync.dma_start(b_t[:], p1[kb * 128:(kb + 1) * 128, t0:t0 + tcw])
                o_t = pool.tile([128, tcw], FP16, tag="o")
                nc.vector.tensor_add(o_t[:], a_t[:], b_t[:])
                nc.sync.dma_start(s[kb * 128:(kb + 1) * 128, t0:t0 + tcw], o_t[:])
    nc.compile()
    return nc


def _stage1_inputs(x, A_log, Dp, W_xproj, b_xproj, W_out, b_out):
    bf = ml_dtypes.bfloat16
    in_maps = []
    for c in range(8):
        b, j = c % 4, c // 4
        lo, hi = j * DL, (j + 1) * DL
        order = np.concatenate(
            [np.arange(lo, hi), np.arange(0, lo), np.arange(hi, D)])
        cols = np.concatenate([np.arange(lo, hi), np.arange(D, D + 2 * N)])
        xt_full = np.ascontiguousarray(x[b].T[order]).astype(bf)
        wxc = np.ascontiguousarray(W_xproj[order][:, cols]).astype(bf)
        bx_pad = np.zeros(5 * 128, np.float32)
        bx_pad[:PC] = b_xproj[cols]
        bx_arr = np.ascontiguousarray(bx_pad.reshape(5, 128).T)
        alog_l = np.ascontiguousarray(
            A_log[lo:hi].reshape(NJB, 128, N).transpose(1, 0, 2).reshape(128, NJB * N))
        dpd = np.zeros((NJB * 128, 128), np.float32)
        for jb in range(NJB):
            np.fill_diagonal(dpd[jb * 128:(jb + 1) * 128],
                             Dp[lo + jb * 128:lo + (jb + 1) * 128])
        wo_l = np.ascontiguousarray(W_out[lo:hi]).astype(bf)
        bo_src = b_out if j == 0 else np.zeros_like(b_out)
        bo_l = np.ascontiguousarray(bo_src.reshape(NEB, 128).T.astype(np.float32))
        in_maps.append({
            "xt": xt_full, "wx": wxc, "bx": bx_arr, "alog": alog_l,
            "dpdiag": dpd.astype(bf), "wo": wo_l, "bo": bo_l,
            "ident": np.eye(128, dtype=bf),
        })
    return in_maps


def kernel(x, A_log, Dp, W_xproj, b_xproj, W_out, b_out, _trace=False):
    x = np.asarray(x, np.float32)
    A_log = np.asarray(A_log, np.float32)
    Dp = np.asarray(Dp, np.float32)
    W_xproj = np.asarray(W_xproj, np.float32)
    b_xproj = np.asarray(b_xproj, np.float32)
    W_out = np.asarray(W_out, np.float32)
    b_out = np.asarray(b_out, np.float32)

    if "s1" not in _cache:
        _cache["s1"] = _build_stage1()
    if "s2" not in _cache:
        _cache["s2"] = _build_stage2()

    in1 = _stage1_inputs(x, A_log, Dp, W_xproj, b_xproj, W_out, b_out)
    kw = dict(trace=True, trace_cores=list(range(8))) if _trace else {}
    res1 = run_bass_kernel_spmd(_cache["s1"], in1, core_ids=list(range(8)), **kw)
    # part is all-reduced across each (b, b+4) core pair on-device
    outs = [res1.results[b]["part"].T for b in range(4)]
    out = np.stack(outs).astype(np.float32)
    if _trace:
        return out, (res1,)
    return out

